# revision 1
# baseline (speedup 1.0000x reference)
"""PointTransformerLayer Bass kernel for TRN2.

Design (per core, points sharded across 8 cores):
  - Neighbor data comes from a packed DRAM table: one 512B row per point =
    [feats fp16 (128) | a fp16 (3) | zero pad], where a = pos @ Afold with the
    first linear_p layer + BN folded in (host-side parameter folding).
  - dma_gather int16 indices only reach 32768 rows, so the table is split
    lo/hi with a zero row at index 0 of each; out-of-range indices map to the
    zero row and the two gathered tiles are merged with one DVE add.
  - Transpose-mode gather lands channels on partitions: G[c, (pt,k)] -- the
    layout every downstream matmul wants.
  - All BN layers are affine in eval mode and folded into weights/per-channel
    biases (applied via ACT per-partition scale/bias).
  - Softmax logits are computed with Ww2 column-tiled 8x so exp() output IS
    the [128, n] broadcast weight tile; sum/normalize happen post-reduction.
  - Residual + leaky_relu in two small DVE ops; output written transposed and
    fixed up on the host.
"""

import sys

sys.path.insert(0, "/opt/trn_rl_repo")
sys.path.insert(0, "/root/.axon_site/_ro/trn_rl_repo")

import numpy as np

import concourse.bass as bass
import concourse.tile as tile
from concourse import library_config, mybir

F16 = mybir.dt.float16
F32 = mybir.dt.float32
I16 = mybir.dt.int16

K = 16
C = 128
S = 8
CS = C // S  # 16
EPS = 1e-5
EXP_SHIFT = float(np.log(256.0))  # subtracted from logits before exp
PT_TILE = 128          # points per tile
NPAIR = PT_TILE * K    # 2048 gather columns per tile
CHUNK = 1024           # psum column chunk
QHI = 0                # hw: only swdge queue 0 is serviced
MM = 512               # moving-operand columns per matmul


# ----------------------------------------------------------------- host math
def fold_params(p):
    """Fold BN params / biases. Tiny O(C^2) parameter-only preprocessing."""
    f32 = np.float32
    s_p = (p["p_gamma"] / np.sqrt(p["p_var"] + EPS)).astype(f32)
    Afold = (p["Wp1"] * s_p[None, :]).astype(f32)
    cfold = ((p["bp1"] - p["p_mean"]) * s_p + p["p_beta"]).astype(f32)

    s_w = (p["w_gamma"] / np.sqrt(p["w_var"] + EPS)).astype(f32)
    ball = (p["bk"] - p["bq"] + p["bp2"]).astype(f32)
    b_w = ((ball - p["w_mean"]) * s_w + p["w_beta"]).astype(f32)

    s1 = (p["w1_gamma"] / np.sqrt(p["w1_var"] + EPS)).astype(f32)
    ww1s = (p["Ww1"] * s1[None, :]).astype(np.float16)
    b1f = ((p["bw1"] - p["w1_mean"]) * s1 + p["w1_beta"]).astype(f32)

    ww2r = np.tile(p["Ww2"], (1, S)).astype(np.float16)          # [16, 128]
    be_bias = (np.tile(p["bw2"], S) - EXP_SHIFT).astype(f32)      # [128]
    bvp = (p["bv"] + p["bp2"]).astype(f32)                        # [128]

    return dict(
        wk=p["Wk"].astype(np.float16),
        wv=p["Wv"].astype(np.float16),
        wqn=(-p["Wq"]).astype(np.float16),
        wp2=p["Wp2"].astype(np.float16),  # [3, 128]
        ww1s=ww1s, ww2r=ww2r,
        s_w=s_w, b_w=b_w, b1f=b1f, be_bias=be_bias, bvp=bvp,
        Afold=Afold, cfold=cfold,
    )


def prep_inputs(xyz, feats, nei_ind, params, n_cores, thresh):
    """Build per-core in_maps. Host work is slicing / transposes / dtype
    conversion plus the tiny parameter folds above."""
    f = fold_params(params)
    n_real = feats.shape[1]
    per_core_raw = -(-n_real // n_cores)
    per_core = -(-per_core_raw // PT_TILE) * PT_TILE
    npad = per_core * n_cores
    n_tiles = per_core // PT_TILE

    feats0 = np.zeros((npad, C), np.float32)
    feats0[:n_real] = feats[0]
    pos0 = np.zeros((npad, 3), np.float32)
    pos0[:n_real] = xyz[0]
    ni = np.zeros((npad, K), np.int64)
    ni[:n_real] = nei_ind[0]

    a = (pos0 @ f["Afold"]).astype(np.float32)            # [npad, 3]
    actrC = (a - f["cfold"][None, :]).astype(np.float32)  # center role

    # packed table rows: [feats f16 (128) | a f16 (3) | pad] = 256 f16 = 512B
    ent = np.zeros((npad, 256), np.float16)
    ent[:, :C] = feats0.astype(np.float16)
    ent[:, C:C + 3] = a.astype(np.float16)

    lo_rows = thresh + 1
    hi_rows = npad - thresh + 1
    table_lo = np.zeros((lo_rows, 256), np.float16)
    table_lo[1:] = ent[:thresh]
    table_hi = np.zeros((hi_rows, 256), np.float16)
    table_hi[1:] = ent[thresh:]

    lo_all = np.where(ni < thresh, ni + 1, 0).astype(np.int16)        # [npad, K]
    hi_all = np.where(ni >= thresh, ni - thresh + 1, 0).astype(np.int16)

    featsT = np.ascontiguousarray(feats0.T)               # [C, npad] f32
    actrT = np.ascontiguousarray(actrC.T)                 # [3, npad] f32

    def wrap_idx(arr_core):
        # arr_core: [per_core, K] -> [128, n_tiles*128] int16 in the
        # (s p)-wrapped layout dma_gather expects, replicated to 8 groups.
        out = np.zeros((128, n_tiles * 128), np.int16)
        for t in range(n_tiles):
            flat = arr_core[t * PT_TILE:(t + 1) * PT_TILE].reshape(-1)  # 2048
            w16 = flat.reshape(128, 16).T                                # [16,128]
            out[:, t * 128:(t + 1) * 128] = np.tile(w16, (8, 1))
        return out

    in_maps = []
    for c in range(n_cores):
        sl = slice(c * per_core, (c + 1) * per_core)
        actrE = np.repeat(actrT[:, sl].astype(np.float16), K, axis=1)  # [3, per_core*K]
        in_maps.append({
            "table_lo": table_lo, "table_hi": table_hi,
            "idx_lo": wrap_idx(lo_all[sl]), "idx_hi": wrap_idx(hi_all[sl]),
            "featsT": np.ascontiguousarray(featsT[:, sl]),
            "actrE": np.ascontiguousarray(actrE),
            "wk": f["wk"], "wv": f["wv"], "wqn": f["wqn"], "wp2": f["wp2"],
            "ww1s": f["ww1s"], "ww2r": f["ww2r"],
            "s_w": f["s_w"].reshape(C, 1), "b_w": f["b_w"].reshape(C, 1),
            "b1f": f["b1f"].reshape(CS, 1),
            "be_bias": f["be_bias"].reshape(C, 1),
            "bvp": f["bvp"].reshape(C, 1),
        })
    meta = dict(n_tiles=n_tiles, per_core=per_core, npad=npad,
                lo_rows=lo_rows, hi_rows=hi_rows, n_real=n_real)
    return in_maps, meta


# ------------------------------------------------------------- walrus compat
def split_excess_waits(nc, max_waits=1):
    """This walrus build allows only 1 sync wait on CTRL instructions
    (Drain/NoOp) and a few on compute instructions. Move excess waits onto
    preceding single-wait NoOps."""
    n_split = 0
    for fn in nc.m.functions:
        for blk in fn.blocks:
            new_insts = []
            for inst in blk.instructions:
                si = inst.sync_info
                lim = (1 if isinstance(inst, (mybir.InstDrain, mybir.InstNoOp,
                                              mybir.InstEventSemaphore))
                       else max_waits)
                if si is not None and si.on_wait and len(si.on_wait) > lim:
                    waits = list(si.on_wait)
                    extra, keep = waits[:-lim], waits[-lim:]
                    ci = 0
                    while extra:
                        chunk, extra = extra[:1], extra[1:]
                        new_insts.append(mybir.InstNoOp(
                            name=f"{inst.name}-waitsplit{ci}",
                            engine=inst.engine,
                            bass_nofuse=True,
                            sync_info=mybir.SyncInfo(on_wait=chunk, on_update=[]),
                        ))
                        ci += 1
                    si.on_wait = keep
                    n_split += 1
                new_insts.append(inst)
            blk.instructions = new_insts
    return n_split


# ----------------------------------------------------------------- the kernel
def build_nc(meta, enable_asserts=False, pe_bcast=True, split_waits=True):
    n_tiles = meta["n_tiles"]
    per_core = meta["per_core"]
    nc = bass.Bass("TRN2", target_bir_lowering=False, debug=False,
                   enable_asserts=enable_asserts, num_swdge_queues=1)

    dt_ = nc.dram_tensor
    t_lo = dt_("table_lo", [meta["lo_rows"], 256], F16, kind="ExternalInput").ap()
    t_hi = dt_("table_hi", [meta["hi_rows"], 256], F16, kind="ExternalInput").ap()
    idx_lo = dt_("idx_lo", [128, n_tiles * 128], I16, kind="ExternalInput").ap()
    idx_hi = dt_("idx_hi", [128, n_tiles * 128], I16, kind="ExternalInput").ap()
    featsT = dt_("featsT", [C, per_core], F32, kind="ExternalInput").ap()
    actrE = dt_("actrE", [3, per_core * K], F16, kind="ExternalInput").ap()
    wk_d = dt_("wk", [C, C], F16, kind="ExternalInput").ap()
    wv_d = dt_("wv", [C, C], F16, kind="ExternalInput").ap()
    wqn_d = dt_("wqn", [C, C], F16, kind="ExternalInput").ap()
    wp2_d = dt_("wp2", [3, C], F16, kind="ExternalInput").ap()
    ww1s_d = dt_("ww1s", [C, CS], F16, kind="ExternalInput").ap()
    ww2r_d = dt_("ww2r", [CS, C], F16, kind="ExternalInput").ap()
    s_w_d = dt_("s_w", [C, 1], F32, kind="ExternalInput").ap()
    b_w_d = dt_("b_w", [C, 1], F32, kind="ExternalInput").ap()
    b1f_d = dt_("b1f", [CS, 1], F32, kind="ExternalInput").ap()
    be_d = dt_("be_bias", [C, 1], F32, kind="ExternalInput").ap()
    bvp_d = dt_("bvp", [C, 1], F32, kind="ExternalInput").ap()
    outT = dt_("outT", [C, per_core], F32, kind="ExternalOutput").ap()

    Relu = mybir.ActivationFunctionType.Relu
    Exp = mybir.ActivationFunctionType.Exp
    ADD = mybir.AluOpType.add
    MULT = mybir.AluOpType.mult
    SUB = mybir.AluOpType.subtract
    MAX = mybir.AluOpType.max

    nc.gpsimd.load_library(library_config.mlp)
    nidx_reg = nc.gpsimd.alloc_register("nidx")
    nc.gpsimd.reg_mov(nidx_reg, NPAIR)

    with tile.TileContext(nc) as tc:
        with (
            tc.tile_pool(name="const", bufs=1) as cpool,
            tc.tile_pool(name="gath", bufs=2) as gpool,
            tc.tile_pool(name="gm", bufs=2) as gmpool,
            tc.tile_pool(name="xs", bufs=2) as xpool,
            tc.tile_pool(name="mid", bufs=2) as mpool,
            tc.tile_pool(name="tail", bufs=2) as tpool,
            tc.tile_pool(name="psA", bufs=4, space="PSUM") as psA,
        ):
            # ---- constants into SBUF once
            def cload(ap_dram, shape, dtype, tag):
                t = cpool.tile(shape, dtype, tag=tag)
                nc.sync.dma_start(t[:], ap_dram)
                return t

            wk = cload(wk_d, [C, C], F16, "wk")
            wv = cload(wv_d, [C, C], F16, "wv")
            wqn = cload(wqn_d, [C, C], F16, "wqn")
            wp2 = cload(wp2_d, [3, C], F16, "wp2")
            ww1s = cload(ww1s_d, [C, CS], F16, "ww1s")
            ww2r = cload(ww2r_d, [CS, C], F16, "ww2r")
            s_w = cload(s_w_d, [C, 1], F32, "s_w")
            b_w = cload(b_w_d, [C, 1], F32, "b_w")
            b1f = cload(b1f_d, [CS, 1], F32, "b1f")
            be_b = cload(be_d, [C, 1], F32, "be_b")
            bvp = cload(bvp_d, [C, 1], F32, "bvp")
            ixlo = cload(idx_lo, [128, n_tiles * 128], I16, "ixlo")
            ixhi = cload(idx_hi, [128, n_tiles * 128], I16, "ixhi")

            # whole-core featsT resident in SBUF (one DMA, 20KB/partition)
            ftw = cpool.tile([C, per_core], F32, tag="ftw")
            nc.sync.dma_start(ftw[:], featsT)

            ACHUNK = 2  # tiles per actrE load
            act_ch = None

            state = {}

            def s0_gather(t):
                cols = bass.ts(t, 128)
                glo = gpool.tile([128, 2, NPAIR], F16, tag="glo")
                nc.gpsimd.dma_gather(glo[:], t_lo, ixlo[:, cols], NPAIR, nidx_reg,
                                     256, transpose=True, queue_num=0,
                                     single_packet=False)
                ghi = gpool.tile([128, 2, NPAIR], F16, tag="ghi")
                nc.gpsimd.dma_gather(ghi[:], t_hi, ixhi[:, cols], NPAIR, nidx_reg,
                                     256, transpose=True, queue_num=QHI,
                                     single_packet=False)
                if t % ACHUNK == 0:
                    nch = min(ACHUNK, n_tiles - t)
                    act_ch = xpool.tile([3, ACHUNK * NPAIR], F16, tag="act")
                    nc.sync.dma_start(act_ch[:, :nch * NPAIR],
                                      actrE[:, t * NPAIR:(t + nch) * NPAIR])
                    state["act_ch"] = act_ch
                state[("g", t)] = (glo, ghi, state["act_ch"])

            def s1_front(t):
                glo, ghi, act_ch = state.pop(("g", t))
                act = act_ch[:, (t % ACHUNK) * NPAIR:(t % ACHUNK + 1) * NPAIR]
                gm = gmpool.tile([128, 2, NPAIR], F16, tag="gm")
                nc.vector.tensor_tensor(gm[:], glo[:], ghi[:], ADD)
                xT16 = xpool.tile([C, 128], F16, tag="xT16")
                nc.vector.tensor_copy(xT16[:], ftw[:, bass.ts(t, 128)])
                u = mpool.tile([3, NPAIR], F16, tag="u")
                nc.vector.scalar_tensor_tensor(u[:], gm[0:3, 1, :], 0.0, act,
                                               ADD, SUB)
                ru = mpool.tile([3, NPAIR], F16, tag="ru")
                nc.vector.tensor_scalar(ru[:], u[:], 0.0, None, MAX)
                state[("f", t)] = (gm, xT16, ru)

            def s2_chunks(t):
                gm, xT16, ru = state.pop(("f", t))
                gf = gm[:, 0, :]
                r_t = mpool.tile([C, NPAIR], F16, tag="r")
                h2_t = mpool.tile([CS, NPAIR], F16, tag="h2")
                e_t = mpool.tile([C, NPAIR], F16, tag="e")
                t2_t = mpool.tile([C, NPAIR], F16, tag="t2")
                v16_t = mpool.tile([C, NPAIR], F16, tag="v16")
                nmm = CHUNK // MM

                def gf_sl(lo, n):
                    return gf[:, lo:lo + n]

                def ru_sl(lo, n):
                    return ru[:, lo:lo + n]

                def q_sl(lo, n):
                    p0 = lo // K
                    return (xT16[:, p0:p0 + n // K]
                            .unsqueeze(2).broadcast_to([C, n // K, K]))

                for ch in range(NPAIR // CHUNK):
                    csl = bass.ts(ch, CHUNK)
                    wps = psA.tile([C, CHUNK], F32, tag="big")
                    specs_w = [(wk[:], gf_sl), (wqn[:], q_sl), (wp2[:], ru_sl)]
                    for wi, (lhsT, rfn) in enumerate(specs_w):
                        for hf in range(nmm):
                            lo = ch * CHUNK + hf * MM
                            nc.tensor.matmul(
                                wps[:, hf * MM:(hf + 1) * MM], lhsT, rfn(lo, MM),
                                start=(wi == 0), stop=(wi == len(specs_w) - 1))
                    nc.scalar.activation(r_t[:, csl], wps[:], Relu,
                                         bias=b_w[:], scale=s_w[:])
                    hps_full = psA.tile([C, CHUNK], F32, tag="big")
                    hps = hps_full[0:CS, :]
                    for hf in range(nmm):
                        lo = ch * CHUNK + hf * MM
                        nc.tensor.matmul(hps[:, hf * MM:(hf + 1) * MM], ww1s[:],
                                         r_t[:, lo:lo + MM],
                                         start=True, stop=True)
                    nc.scalar.activation(h2_t[:, csl], hps[:], Relu, bias=b1f[:])
                    lps = psA.tile([C, CHUNK], F32, tag="big")
                    for hf in range(nmm):
                        lo = ch * CHUNK + hf * MM
                        nc.tensor.matmul(lps[:, hf * MM:(hf + 1) * MM], ww2r[:],
                                         h2_t[:, lo:lo + MM],
                                         start=True, stop=True)
                    nc.scalar.activation(e_t[:, csl], lps[:], Exp, bias=be_b[:])
                    vps = psA.tile([C, CHUNK], F32, tag="big")
                    specs_v = [(wv[:], gf_sl), (wp2[:], ru_sl)]
                    for wi, (lhsT, rfn) in enumerate(specs_v):
                        for hf in range(nmm):
                            lo = ch * CHUNK + hf * MM
                            nc.tensor.matmul(
                                vps[:, hf * MM:(hf + 1) * MM], lhsT, rfn(lo, MM),
                                start=(wi == 0), stop=(wi == len(specs_v) - 1))
                    nc.scalar.copy(v16_t[:, csl], vps[:])
                    nc.vector.tensor_tensor(t2_t[:, csl], e_t[:, csl],
                                            v16_t[:, csl], MULT)
                state[("c", t)] = (e_t, t2_t)

            def s3_tail(t):
                e_t, t2_t = state.pop(("c", t))

                def ktree(src_t, out32, tagp):
                    cur = src_t[:].rearrange("p (a b) -> p a b", b=K)
                    kk = K
                    while kk > 2:
                        nx = tpool.tile([C, 128 * kk // 2], F16, tag=f"{tagp}{kk}")
                        nxv = nx[:].rearrange("p (a b) -> p a b", b=kk // 2)
                        nc.vector.tensor_tensor(
                            nxv, cur[:, :, 0:kk // 2], cur[:, :, kk // 2:kk], ADD)
                        cur, kk = nxv, kk // 2
                    nc.vector.tensor_tensor(out32[:], cur[:, :, 0], cur[:, :, 1], ADD)

                S_t = tpool.tile([C, 128], F32, tag="S")
                ktree(e_t, S_t, "se")
                aggU = tpool.tile([C, 128], F32, tag="aggU")
                ktree(t2_t, aggU, "sa")
                rS = tpool.tile([C, 128], F32, tag="rS")
                nc.vector.reciprocal(rS[:], S_t[:])
                aggN = tpool.tile([C, 128], F32, tag="aggN")
                nc.vector.tensor_tensor(aggN[:], aggU[:], rS[:], MULT)
                l1 = tpool.tile([C, 128], F32, tag="l1")
                nc.vector.scalar_tensor_tensor(l1[:], aggN[:], bvp[:],
                                               ftw[:, bass.ts(t, 128)], ADD, ADD)
                outc = tpool.tile([C, 128], F32, tag="outc")
                nc.vector.scalar_tensor_tensor(outc[:], l1[:], 0.1, l1[:],
                                               MULT, MAX)
                nc.sync.dma_start(outT[:, bass.ts(t, 128)], outc[:])

            for i in range(n_tiles + 3):
                if i < n_tiles:
                    s0_gather(i)
                if 1 <= i < n_tiles + 1:
                    s1_front(i - 1)
                if 2 <= i < n_tiles + 2:
                    s2_chunks(i - 2)
                if 3 <= i:
                    s3_tail(i - 3)

    from concourse.library_overlay import lower_extended_insts
    lower_extended_insts(nc)
    if split_waits:
        split_excess_waits(nc)
    return nc




# ------------------------------------------------------------- entry point
N_CORES = 8
THRESH = 32767  # int16 row-index reach (with +1 zero-row offset)

_CACHE = {}


def kernel(**inputs) -> np.ndarray:
    """Full-input entry: shards points across 8 NeuronCores, runs the Bass
    kernel via run_bass_kernel_spmd, reassembles the full (1, N, C) output."""
    from concourse.bass_utils import run_bass_kernel_spmd

    xyz = np.asarray(inputs["xyz"], np.float32)
    feats = np.asarray(inputs["feats"], np.float32)
    nei = np.asarray(inputs["nei_ind"])
    params = {k: np.asarray(v, np.float32) for k, v in inputs.items()
              if k not in ("xyz", "feats", "nei_ind")}

    in_maps, meta = prep_inputs(xyz, feats, nei, params, N_CORES, THRESH)

    key = (meta["n_tiles"], meta["per_core"], meta["lo_rows"], meta["hi_rows"])
    if key not in _CACHE:
        _CACHE[key] = build_nc(meta)
    nc = _CACHE[key]

    res = run_bass_kernel_spmd(nc, in_maps, core_ids=list(range(N_CORES)))
    outs = [r["outT"] for r in res.results]          # each [C, per_core] f32
    full = np.concatenate(outs, axis=1).T             # [npad, C]
    return np.ascontiguousarray(full[None, :meta["n_real"]]).astype(np.float32)



# revision 56
# speedup vs baseline: 1.4620x; 1.4620x over previous
"""PointTransformerLayer Bass kernel for TRN2 (v4).

Design (per core, points sharded across 8 cores):
  - Packed DRAM table: one 512B row per point = [feats fp16 (128) | a fp16 (3)
    | zero pad], a = pos @ Afold (linear_p layer 1 + BN folded host-side).
  - SINGLE gather per tile: the gather's int16 indices are signed; the table
    base AP is offset to row CO=16384 so idx = j - CO spans [-16384, 24575],
    covering all 40960 rows in one gather (the gpsimd ucode sign-extends
    indices and the 32-bit address mul wraps correctly because the AP offset
    guarantees no underflow below the tensor base). The ucode trims trailing
    NEGATIVE indices, so the host guarantees each tile's last pair has
    idx >= 0 via a slot swap (or, in the astronomically rare fallback, by
    duplicating the needed row into a padding row >= CO).
  - Transpose-mode gather lands channels on partitions: G[c, (pt,k)].
  - All BN folded into weights / per-channel ACT scale+bias.
  - h-path (C -> C/S): single [16,CHUNK] matmul into the dead w-psum bank,
    relu+bias on ACT; single l-matmul per chunk (matmul inputs must sit at
    partition base 0 on this HW).
  - CHUNK=512, psum regions w(3)/v(2)/l(2) bufs, chunks software-pipelined;
    exp output IS the broadcast weight tile; t2 = e * vpsum in one DVE op;
    e16/t2 share one [C,2,NPAIR] slab so both K-reduction trees run as
    single DVE ops per level (16->8->4->2->1, f16 until the last level).
"""

import sys

sys.path.insert(0, "/opt/trn_rl_repo")
sys.path.insert(0, "/root/.axon_site/_ro/trn_rl_repo")

import numpy as np

import concourse.bass as bass
import concourse.tile as tile
from concourse import library_config, mybir

F16 = mybir.dt.float16
F32 = mybir.dt.float32
I16 = mybir.dt.int16

K = 16
C = 128
S = 8
CS = C // S  # 16
EPS = 1e-5
EXP_SHIFT = float(np.log(256.0))
PT_TILE = 128          # points per tile
NPAIR = PT_TILE * K    # 2048 gather columns per tile
CHUNK = 512            # psum column chunk (1 bank)
NCH = NPAIR // CHUNK   # 4 chunks per tile
HB = 4                 # h-blocks per chunk, at PE quadrant bases 0/32/64/96
HCOL = CHUNK // HB     # 128 pair-columns per h-block
CO = 16384             # table base-row offset: idx = j - CO (signed int16)


# ----------------------------------------------------------------- host math
def fold_params(p):
    """Fold BN params / biases. Tiny O(C^2) parameter-only preprocessing."""
    f32 = np.float32
    s_p = (p["p_gamma"] / np.sqrt(p["p_var"] + EPS)).astype(f32)
    Afold = (p["Wp1"] * s_p[None, :]).astype(f32)
    cfold = ((p["bp1"] - p["p_mean"]) * s_p + p["p_beta"]).astype(f32)

    s_w = (p["w_gamma"] / np.sqrt(p["w_var"] + EPS)).astype(f32)
    ball = (p["bk"] - p["bq"] + p["bp2"]).astype(f32)
    b_w = ((ball - p["w_mean"]) * s_w + p["w_beta"]).astype(f32)

    s1 = (p["w1_gamma"] / np.sqrt(p["w1_var"] + EPS)).astype(f32)
    ww1s = (p["Ww1"] * s1[None, :]).astype(np.float16)
    b1f = ((p["bw1"] - p["w1_mean"]) * s1 + p["w1_beta"]).astype(f32)

    ww2r = np.tile(p["Ww2"], (1, S)).astype(np.float16)          # [16, 128]
    be_bias = (np.tile(p["bw2"], S) - EXP_SHIFT).astype(f32)      # [128]
    bvp = (p["bv"] + p["bp2"]).astype(f32)                        # [128]

    return dict(
        wk=p["Wk"].astype(np.float16),
        wv=p["Wv"].astype(np.float16),
        wqn=(-p["Wq"]).astype(np.float16),
        wp2=p["Wp2"].astype(np.float16),  # [3, 128]
        ww1s=ww1s, ww2r=ww2r,
        s_w=s_w, b_w=b_w, b1f=b1f, be_bias=be_bias, bvp=bvp,
        Afold=Afold, cfold=cfold,
    )


def prep_inputs(xyz, feats, nei_ind, params, n_cores):
    """Build per-core in_maps. Host work is slicing / transposes / dtype
    conversion plus the tiny parameter folds above."""
    f = fold_params(params)
    n_real = feats.shape[1]
    per_core_raw = -(-n_real // n_cores)
    per_core = -(-per_core_raw // PT_TILE) * PT_TILE
    npad = per_core * n_cores
    n_tiles = per_core // PT_TILE

    feats0 = np.zeros((npad, C), np.float32)
    feats0[:n_real] = feats[0]
    pos0 = np.zeros((npad, 3), np.float32)
    pos0[:n_real] = xyz[0]
    ni = np.full((npad, K), CO, np.int64)   # padding points gather row CO
    ni[:n_real] = nei_ind[0]

    a = (pos0 @ f["Afold"]).astype(np.float32)            # [npad, 3]
    actrC = (a - f["cfold"][None, :]).astype(np.float32)  # center role

    # packed table rows: [feats f16 (128) | a f16 (3) | pad] = 256 f16 = 512B
    ent = np.zeros((npad, 256), np.float16)
    ent[:, :C] = feats0.astype(np.float16)
    ent[:, C:C + 3] = a.astype(np.float16)

    featsT = np.ascontiguousarray(feats0.T)               # [C, npad] f32
    actrT = np.ascontiguousarray(actrC.T)                 # [3, npad] f32

    def wrap_idx(arr_core):
        # arr_core: [per_core, K] int16 -> [128, n_tiles*128] in the
        # (s p)-wrapped layout dma_gather expects, replicated to 8 groups.
        out = np.zeros((128, n_tiles * 128), np.int16)
        for t in range(n_tiles):
            flat = arr_core[t * PT_TILE:(t + 1) * PT_TILE].reshape(-1)  # 2048
            w16 = flat.reshape(128, 16).T                                # [16,128]
            out[:, t * 128:(t + 1) * 128] = np.tile(w16, (8, 1))
        return out

    in_maps = []
    n_pad_rows = npad - n_real
    for c in range(n_cores):
        sl = slice(c * per_core, (c + 1) * per_core)
        ni_c = ni[sl].copy()                              # [per_core, K]
        table_c = ent                                     # shared unless patched
        # the gather ucode trims trailing negative indices: guarantee the
        # last pair of every tile has j >= CO (idx >= 0)
        for t in range(n_tiles):
            row = ni_c[t * PT_TILE + PT_TILE - 1]
            if row[K - 1] >= CO:
                continue
            hi = np.nonzero(row >= CO)[0]
            if len(hi):
                row[K - 1], row[hi[0]] = row[hi[0]], row[K - 1]
            else:
                # astronomically rare: duplicate the needed row into a
                # padding slot >= CO and retarget the index
                pr = n_real + (c * n_tiles + t) % n_pad_rows
                if table_c is ent:
                    table_c = ent.copy()
                table_c[pr] = ent[row[K - 1]]
                row[K - 1] = pr
        idxs = (ni_c - CO).astype(np.int16)
        actrE = np.repeat(actrT[:, sl].astype(np.float16), K, axis=1)  # [3, per_core*K]
        in_maps.append({
            "table": table_c,
            "idxs": wrap_idx(idxs),
            "featsT": np.ascontiguousarray(featsT[:, sl]),
            "actrE": np.ascontiguousarray(actrE),
            "wk": f["wk"], "wv": f["wv"], "wqn": f["wqn"], "wp2": f["wp2"],
            "ww1s": f["ww1s"], "ww2r": f["ww2r"],
            "s_w": f["s_w"].reshape(C, 1), "b_w": f["b_w"].reshape(C, 1),
            "b1f": f["b1f"].reshape(CS, 1),
            "be_bias": f["be_bias"].reshape(C, 1),
            "bvp": f["bvp"].reshape(C, 1),
        })
    meta = dict(n_tiles=n_tiles, per_core=per_core, npad=npad, n_real=n_real)
    return in_maps, meta


# ------------------------------------------------------------- walrus compat
def split_excess_waits(nc, max_waits=1):
    """This walrus build allows only 1 sync wait on CTRL instructions
    (Drain/NoOp) and a few on compute instructions. Move excess waits onto
    preceding single-wait NoOps."""
    n_split = 0
    for fn in nc.m.functions:
        for blk in fn.blocks:
            new_insts = []
            for inst in blk.instructions:
                si = inst.sync_info
                lim = (1 if isinstance(inst, (mybir.InstDrain, mybir.InstNoOp,
                                              mybir.InstEventSemaphore))
                       else max_waits)
                if si is not None and si.on_wait and len(si.on_wait) > lim:
                    waits = list(si.on_wait)
                    extra, keep = waits[:-lim], waits[-lim:]
                    ci = 0
                    while extra:
                        chunk, extra = extra[:1], extra[1:]
                        new_insts.append(mybir.InstNoOp(
                            name=f"{inst.name}-waitsplit{ci}",
                            engine=inst.engine,
                            bass_nofuse=True,
                            sync_info=mybir.SyncInfo(on_wait=chunk, on_update=[]),
                        ))
                        ci += 1
                    si.on_wait = keep
                    n_split += 1
                new_insts.append(inst)
            blk.instructions = new_insts
    return n_split


# ----------------------------------------------------------------- the kernel
def build_nc(meta, enable_asserts=False, split_waits=True):
    n_tiles = meta["n_tiles"]
    per_core = meta["per_core"]
    npad = meta["npad"]
    nc = bass.Bass("TRN2", target_bir_lowering=False, debug=False,
                   enable_asserts=enable_asserts, num_swdge_queues=1)

    dt_ = nc.dram_tensor
    t_tab = dt_("table", [npad, 256], F16, kind="ExternalInput").ap()
    idxs_d = dt_("idxs", [128, n_tiles * 128], I16, kind="ExternalInput").ap()
    featsT = dt_("featsT", [C, per_core], F32, kind="ExternalInput").ap()
    actrE = dt_("actrE", [3, per_core * K], F16, kind="ExternalInput").ap()
    wk_d = dt_("wk", [C, C], F16, kind="ExternalInput").ap()
    wv_d = dt_("wv", [C, C], F16, kind="ExternalInput").ap()
    wqn_d = dt_("wqn", [C, C], F16, kind="ExternalInput").ap()
    wp2_d = dt_("wp2", [3, C], F16, kind="ExternalInput").ap()
    ww1s_d = dt_("ww1s", [C, CS], F16, kind="ExternalInput").ap()
    ww2r_d = dt_("ww2r", [CS, C], F16, kind="ExternalInput").ap()
    s_w_d = dt_("s_w", [C, 1], F32, kind="ExternalInput").ap()
    b_w_d = dt_("b_w", [C, 1], F32, kind="ExternalInput").ap()
    b1f_d = dt_("b1f", [CS, 1], F32, kind="ExternalInput").ap()
    be_d = dt_("be_bias", [C, 1], F32, kind="ExternalInput").ap()
    bvp_d = dt_("bvp", [C, 1], F32, kind="ExternalInput").ap()
    outT = dt_("outT", [C, per_core], F32, kind="ExternalOutput").ap()

    # gather base AP offset to row CO so signed indices reach the whole table
    t_base = t_tab[CO:npad, :]

    Relu = mybir.ActivationFunctionType.Relu
    Exp = mybir.ActivationFunctionType.Exp
    ADD = mybir.AluOpType.add
    MULT = mybir.AluOpType.mult
    SUB = mybir.AluOpType.subtract
    MAX = mybir.AluOpType.max

    nc.gpsimd.load_library(library_config.mlp)
    nidx_reg = nc.gpsimd.alloc_register("nidx")
    nc.gpsimd.reg_mov(nidx_reg, NPAIR)

    with tile.TileContext(nc) as tc:
        with (
            tc.tile_pool(name="const", bufs=1) as cpool,
            tc.tile_pool(name="gath", bufs=6) as gpool,
            tc.tile_pool(name="xs", bufs=2) as xpool,
            tc.tile_pool(name="front", bufs=2) as fpool,
            tc.tile_pool(name="rr", bufs=4) as rpool,
            tc.tile_pool(name="hh", bufs=4) as hpool,
            tc.tile_pool(name="slab", bufs=2) as epool,
            tc.tile_pool(name="tail", bufs=3) as tpool,
            tc.tile_pool(name="psWL", bufs=2, space="PSUM") as psW,
            tc.tile_pool(name="psV", bufs=3, space="PSUM") as psV,
            tc.tile_pool(name="psL", bufs=2, space="PSUM") as psL,
        ):
            # ---- constants into SBUF once
            def cload(ap_dram, shape, dtype, tag):
                t = cpool.tile(shape, dtype, tag=tag)
                nc.sync.dma_start(t[:], ap_dram)
                return t

            wk = cload(wk_d, [C, C], F16, "wk")
            wv = cload(wv_d, [C, C], F16, "wv")
            wqn = cload(wqn_d, [C, C], F16, "wqn")
            wp2 = cload(wp2_d, [3, C], F16, "wp2")
            ww1s = cload(ww1s_d, [C, CS], F16, "ww1s")
            ww2r = cload(ww2r_d, [CS, C], F16, "ww2r")
            s_w = cload(s_w_d, [C, 1], F32, "s_w")
            b_w = cload(b_w_d, [C, 1], F32, "b_w")
            b1f = cload(b1f_d, [CS, 1], F32, "b1f")
            be_b = cload(be_d, [C, 1], F32, "be_b")
            bvp = cload(bvp_d, [C, 1], F32, "bvp")
            ixs = cload(idxs_d, [128, n_tiles * 128], I16, "ixs")

            # whole-core featsT resident in SBUF (one DMA, 20KB/partition)
            ftw = cpool.tile([C, per_core], F32, tag="ftw")
            nc.sync.dma_start(ftw[:], featsT)

            ACHUNK = 2  # tiles per actrE load
            state = {}

            def s0_gather(t):
                cols = bass.ts(t, 128)
                g = gpool.tile([128, 2, NPAIR], F16, tag="g")
                nc.gpsimd.dma_gather(g[:], t_base, ixs[:, cols], NPAIR, nidx_reg,
                                     256, transpose=True, queue_num=0,
                                     single_packet=False)
                if t % ACHUNK == 0:
                    nch = min(ACHUNK, n_tiles - t)
                    act_ch = xpool.tile([3, ACHUNK * NPAIR], F16, tag="act")
                    nc.sync.dma_start(act_ch[:, :nch * NPAIR],
                                      actrE[:, t * NPAIR:(t + nch) * NPAIR])
                    state["act_ch"] = act_ch
                state[("g", t)] = (g, state["act_ch"])

            def s1_front(t):
                g, act_ch = state[("g", t)]
                act = act_ch[:, (t % ACHUNK) * NPAIR:(t % ACHUNK + 1) * NPAIR]
                xT16 = xpool.tile([C, 128], F16, tag="xT16")
                nc.scalar.copy(xT16[:], ftw[:, bass.ts(t, 128)])
                u = fpool.tile([3, NPAIR], F16, tag="u")
                nc.vector.tensor_tensor(u[:], g[0:3, 1, :], act, SUB)
                ru = fpool.tile([3, NPAIR], F16, tag="ru")
                nc.vector.tensor_scalar(ru[:], u[:], 0.0, None, MAX)
                state[("f", t)] = (xT16, ru)

            def s2_chunks(t):
                g, _ = state.pop(("g", t))
                xT16, ru = state.pop(("f", t))
                gf = g[:, 0, :]
                et = epool.tile([C, 2, NPAIR], F16, tag="et")
                e16 = et[:, 0, :]
                t2 = et[:, 1, :]
                filler = state.pop("tree_ops", [])

                def q_bcast(c):
                    p0 = c * (CHUNK // K)
                    return (xT16[:, p0:p0 + CHUNK // K]
                            .unsqueeze(2).broadcast_to([C, CHUNK // K, K]))

                # chunk pipeline: front(c) then back(c-1).
                # h-matmul [16,CHUNK] into the dead w-bank region; relu+bias
                # via TSP (DVE) or ACT alternating chunks to balance load;
                # all matmul inputs at partition base 0 (HW requirement).
                def front(c):
                    csl = bass.ts(c, CHUNK)
                    wps = psW.tile([C, CHUNK], F32, tag="wl")
                    nc.tensor.matmul(wps[:], wk[:], gf[:, csl],
                                     start=True, stop=False)
                    nc.tensor.matmul(wps[:], wqn[:], q_bcast(c),
                                     start=False, stop=False)
                    nc.tensor.matmul(wps[:], wp2[:], ru[:, csl],
                                     start=False, stop=True)
                    vps = psV.tile([C, CHUNK], F32, tag="v")
                    nc.tensor.matmul(vps[:], wv[:], gf[:, csl],
                                     start=True, stop=False)
                    nc.tensor.matmul(vps[:], wp2[:], ru[:, csl],
                                     start=False, stop=True)
                    r16 = rpool.tile([C, CHUNK], F16, tag="r")
                    nc.scalar.activation(r16[:], wps[:], Relu,
                                         bias=b_w[:], scale=s_w[:])
                    return (wps, vps, r16)

                def back(c, wps, vps, r16):
                    hps = wps[0:CS, :]          # dead w-bank after r-ACT
                    nc.tensor.matmul(hps, ww1s[:], r16[:],
                                     start=True, stop=True,
                                     skip_group_check=True)
                    h2 = hpool.tile([CS, CHUNK], F16, tag="h2")
                    nc.scalar.activation(h2[:], hps, Relu, bias=b1f[:])
                    lps = psL.tile([C, CHUNK], F32, tag="l")
                    nc.tensor.matmul(lps[:], ww2r[:], h2[:],
                                     start=True, stop=True)
                    csl = bass.ts(c, CHUNK)
                    nc.scalar.activation(e16[:, csl], lps[:], Exp,
                                         bias=be_b[:])
                    nc.vector.tensor_tensor(t2[:, csl], e16[:, csl],
                                            vps[:], MULT)
                    if filler:
                        filler.pop(0)()

                frs = []
                for c in range(NCH + 1):
                    if c < NCH:
                        frs.append(front(c))
                    if 1 <= c <= NCH:
                        back(c - 1, *frs[c - 1])
                for op in filler:
                    op()
                state[("c", t)] = et

            def s3_trees(t):
                et = state.pop(("c", t))

                # one K-reduction tree over both planes (e sums -> S,
                # t2 sums -> aggU): 16->8->4->2->1, emitted as two closures
                # so s2 can interleave them into DVE bubbles
                st = {}

                def lv_a():
                    cur = et.rearrange("p q (a b) -> p q a b", b=K)
                    nx = tpool.tile([C, 2, 1024], F16, tag="tr16")
                    nxv = nx[:].rearrange("p q (a b) -> p q a b", b=8)
                    nc.vector.tensor_tensor(nxv, cur[:, :, :, 0:8],
                                            cur[:, :, :, 8:16], ADD)
                    st["c"] = nxv

                def lv_b():
                    cur = st["c"]
                    nx = tpool.tile([C, 2, 512], F16, tag="tr8")
                    nxv = nx[:].rearrange("p q (a b) -> p q a b", b=4)
                    nc.vector.tensor_tensor(nxv, cur[:, :, :, 0:4],
                                            cur[:, :, :, 4:8], ADD)
                    nx2 = tpool.tile([C, 2, 256], F16, tag="tr4")
                    nxv2 = nx2[:].rearrange("p q (a b) -> p q a b", b=2)
                    nc.vector.tensor_tensor(nxv2, nxv[:, :, :, 0:2],
                                            nxv[:, :, :, 2:4], ADD)
                    out32 = tpool.tile([C, 2, 128], F32, tag="trout")
                    nc.vector.tensor_tensor(out32[:], nxv2[:, :, :, 0],
                                            nxv2[:, :, :, 1], ADD)
                    state[("sa", t)] = out32

                state["tree_ops"] = [lv_a, lv_b]

            def s3_norm(t):
                for op in state.pop("tree_ops", []):
                    op()   # leftover tree work if s2 didn't run this iter
                sa = state.pop(("sa", t))
                S_t, aggU = sa[:, 0, :], sa[:, 1, :]
                rS = tpool.tile([C, 128], F32, tag="rS")
                nc.vector.reciprocal(rS[:], S_t)
                aggN = tpool.tile([C, 128], F32, tag="aggN")
                nc.gpsimd.tensor_tensor(aggN[:], aggU[:], rS[:], MULT)
                l1 = tpool.tile([C, 128], F32, tag="l1")
                nc.gpsimd.scalar_tensor_tensor(l1[:], aggN[:], bvp[:],
                                               ftw[:, bass.ts(t, 128)], ADD, ADD)
                outc = tpool.tile([C, 128], F32, tag="outc")
                nc.vector.scalar_tensor_tensor(outc[:], l1[:], 0.1, l1[:],
                                               MULT, MAX)
                nc.sync.dma_start(outT[:, bass.ts(t, 128)], outc[:])

            for i in range(n_tiles + 4):
                if 3 <= i < n_tiles + 3:
                    s3_trees(i - 3)
                if i < n_tiles:
                    s0_gather(i)
                if 1 <= i < n_tiles + 1:
                    s1_front(i - 1)
                if 2 <= i < n_tiles + 2:
                    s2_chunks(i - 2)
                if 4 <= i:
                    s3_norm(i - 4)

    from concourse.library_overlay import lower_extended_insts
    lower_extended_insts(nc)
    if split_waits:
        split_excess_waits(nc)
    return nc


# ------------------------------------------------------------- entry point
N_CORES = 8

_CACHE = {}


def kernel(**inputs) -> np.ndarray:
    """Full-input entry: shards points across 8 NeuronCores, runs the Bass
    kernel via run_bass_kernel_spmd, reassembles the full (1, N, C) output."""
    from concourse.bass_utils import run_bass_kernel_spmd

    xyz = np.asarray(inputs["xyz"], np.float32)
    feats = np.asarray(inputs["feats"], np.float32)
    nei = np.asarray(inputs["nei_ind"])
    params = {k: np.asarray(v, np.float32) for k, v in inputs.items()
              if k not in ("xyz", "feats", "nei_ind")}

    in_maps, meta = prep_inputs(xyz, feats, nei, params, N_CORES)

    key = (meta["n_tiles"], meta["per_core"], meta["npad"])
    if key not in _CACHE:
        _CACHE[key] = build_nc(meta)
    nc = _CACHE[key]

    res = run_bass_kernel_spmd(nc, in_maps, core_ids=list(range(N_CORES)))
    outs = [r["outT"] for r in res.results]          # each [C, per_core] f32
    full = np.concatenate(outs, axis=1).T             # [npad, C]
    return np.ascontiguousarray(full[None, :meta["n_real"]]).astype(np.float32)


# revision 58
# speedup vs baseline: 1.4925x; 1.0208x over previous
"""PointTransformerLayer Bass kernel for TRN2 (v4).

Design (per core, points sharded across 8 cores):
  - Packed DRAM table: one 512B row per point = [feats fp16 (128) | a fp16 (3)
    | zero pad], a = pos @ Afold (linear_p layer 1 + BN folded host-side).
  - SINGLE gather per tile: the gather's int16 indices are signed; the table
    base AP is offset to row CO=16384 so idx = j - CO spans [-16384, 24575],
    covering all 40960 rows in one gather (the gpsimd ucode sign-extends
    indices and the 32-bit address mul wraps correctly because the AP offset
    guarantees no underflow below the tensor base). The ucode trims trailing
    NEGATIVE indices, so the host guarantees each tile's last pair has
    idx >= 0 via a slot swap (or, in the astronomically rare fallback, by
    duplicating the needed row into a padding row >= CO).
  - Transpose-mode gather lands channels on partitions: G[c, (pt,k)].
  - All BN folded into weights / per-channel ACT scale+bias.
  - h-path (C -> C/S): single [16,CHUNK] matmul into the dead w-psum bank,
    relu+bias on ACT; single l-matmul per chunk (matmul inputs must sit at
    partition base 0 on this HW).
  - CHUNK=512, psum regions w(3)/v(2)/l(2) bufs, chunks software-pipelined;
    exp output IS the broadcast weight tile; t2 = e * vpsum in one DVE op;
    e16/t2 share one [C,2,NPAIR] slab so both K-reduction trees run as
    single DVE ops per level (16->8->4->2->1, f16 until the last level).
"""

import sys

sys.path.insert(0, "/opt/trn_rl_repo")
sys.path.insert(0, "/root/.axon_site/_ro/trn_rl_repo")

import numpy as np

import concourse.bass as bass
import concourse.tile as tile
from concourse import library_config, mybir

F16 = mybir.dt.float16
F32 = mybir.dt.float32
I16 = mybir.dt.int16

K = 16
C = 128
S = 8
CS = C // S  # 16
EPS = 1e-5
EXP_SHIFT = float(np.log(256.0))
PT_TILE = 128          # points per tile
NPAIR = PT_TILE * K    # 2048 gather columns per tile
CHUNK = 512            # psum column chunk (1 bank)
NCH = NPAIR // CHUNK   # 4 chunks per tile
HB = 4                 # h-blocks per chunk, at PE quadrant bases 0/32/64/96
HCOL = CHUNK // HB     # 128 pair-columns per h-block
CO = 16384             # table base-row offset: idx = j - CO (signed int16)


# ----------------------------------------------------------------- host math
def fold_params(p):
    """Fold BN params / biases. Tiny O(C^2) parameter-only preprocessing."""
    f32 = np.float32
    s_p = (p["p_gamma"] / np.sqrt(p["p_var"] + EPS)).astype(f32)
    Afold = (p["Wp1"] * s_p[None, :]).astype(f32)
    cfold = ((p["bp1"] - p["p_mean"]) * s_p + p["p_beta"]).astype(f32)

    s_w = (p["w_gamma"] / np.sqrt(p["w_var"] + EPS)).astype(f32)
    ball = (p["bk"] - p["bq"] + p["bp2"]).astype(f32)
    b_w = ((ball - p["w_mean"]) * s_w + p["w_beta"]).astype(f32)

    s1 = (p["w1_gamma"] / np.sqrt(p["w1_var"] + EPS)).astype(f32)
    ww1s = (p["Ww1"] * s1[None, :]).astype(np.float16)
    b1f = ((p["bw1"] - p["w1_mean"]) * s1 + p["w1_beta"]).astype(f32)

    ww2r = np.tile(p["Ww2"], (1, S)).astype(np.float16)          # [16, 128]
    be_bias = (np.tile(p["bw2"], S) - EXP_SHIFT).astype(f32)      # [128]
    bvp = (p["bv"] + p["bp2"]).astype(f32)                        # [128]

    return dict(
        wk=p["Wk"].astype(np.float16),
        wv=p["Wv"].astype(np.float16),
        wqn=(-p["Wq"]).astype(np.float16),
        wp2=p["Wp2"].astype(np.float16),  # [3, 128]
        ww1s=ww1s, ww2r=ww2r,
        s_w=s_w, b_w=b_w, b1f=b1f, be_bias=be_bias, bvp=bvp,
        Afold=Afold, cfold=cfold,
    )


def prep_inputs(xyz, feats, nei_ind, params, n_cores):
    """Build per-core in_maps. Host work is slicing / transposes / dtype
    conversion plus the tiny parameter folds above."""
    f = fold_params(params)
    n_real = feats.shape[1]
    per_core_raw = -(-n_real // n_cores)
    per_core = -(-per_core_raw // PT_TILE) * PT_TILE
    npad = per_core * n_cores
    n_tiles = per_core // PT_TILE

    feats0 = np.zeros((npad, C), np.float32)
    feats0[:n_real] = feats[0]
    pos0 = np.zeros((npad, 3), np.float32)
    pos0[:n_real] = xyz[0]
    ni = np.full((npad, K), CO, np.int64)   # padding points gather row CO
    ni[:n_real] = nei_ind[0]

    a = (pos0 @ f["Afold"]).astype(np.float32)            # [npad, 3]
    actrC = (a - f["cfold"][None, :]).astype(np.float32)  # center role

    # packed table rows: [feats f16 (128) | a f16 (3) | pad] = 256 f16 = 512B
    ent = np.zeros((npad, 256), np.float16)
    ent[:, :C] = feats0.astype(np.float16)
    ent[:, C:C + 3] = a.astype(np.float16)

    featsT = np.ascontiguousarray(feats0.T)               # [C, npad] f32
    actrT = np.ascontiguousarray(actrC.T)                 # [3, npad] f32

    def wrap_idx(arr_core):
        # arr_core: [per_core, K] int16 -> [128, n_tiles*128] in the
        # (s p)-wrapped layout dma_gather expects, replicated to 8 groups.
        out = np.zeros((128, n_tiles * 128), np.int16)
        for t in range(n_tiles):
            flat = arr_core[t * PT_TILE:(t + 1) * PT_TILE].reshape(-1)  # 2048
            w16 = flat.reshape(128, 16).T                                # [16,128]
            out[:, t * 128:(t + 1) * 128] = np.tile(w16, (8, 1))
        return out

    in_maps = []
    n_pad_rows = npad - n_real
    for c in range(n_cores):
        sl = slice(c * per_core, (c + 1) * per_core)
        ni_c = ni[sl].copy()                              # [per_core, K]
        table_c = ent                                     # shared unless patched
        # the gather ucode trims trailing negative indices: guarantee the
        # last pair of every tile has j >= CO (idx >= 0)
        for t in range(n_tiles):
            row = ni_c[t * PT_TILE + PT_TILE - 1]
            if row[K - 1] >= CO:
                continue
            hi = np.nonzero(row >= CO)[0]
            if len(hi):
                row[K - 1], row[hi[0]] = row[hi[0]], row[K - 1]
            else:
                # astronomically rare: duplicate the needed row into a
                # padding slot >= CO and retarget the index
                pr = n_real + (c * n_tiles + t) % n_pad_rows
                if table_c is ent:
                    table_c = ent.copy()
                table_c[pr] = ent[row[K - 1]]
                row[K - 1] = pr
        idxs = (ni_c - CO).astype(np.int16)
        actrE = np.repeat(actrT[:, sl].astype(np.float16), K, axis=1)  # [3, per_core*K]
        in_maps.append({
            "table": table_c,
            "idxs": wrap_idx(idxs),
            "featsT": np.ascontiguousarray(featsT[:, sl]),
            "actrE": np.ascontiguousarray(actrE),
            "wk": f["wk"], "wv": f["wv"], "wqn": f["wqn"], "wp2": f["wp2"],
            "ww1s": f["ww1s"], "ww2r": f["ww2r"],
            "s_w": f["s_w"].reshape(C, 1), "b_w": f["b_w"].reshape(C, 1),
            "b1f": f["b1f"].reshape(CS, 1),
            "be_bias": f["be_bias"].reshape(C, 1),
            "bvp": f["bvp"].reshape(C, 1),
        })
    meta = dict(n_tiles=n_tiles, per_core=per_core, npad=npad, n_real=n_real)
    return in_maps, meta


# ------------------------------------------------------------- walrus compat
def split_excess_waits(nc, max_waits=1):
    """This walrus build allows only 1 sync wait on CTRL instructions
    (Drain/NoOp) and a few on compute instructions. Move excess waits onto
    preceding single-wait NoOps."""
    n_split = 0
    for fn in nc.m.functions:
        for blk in fn.blocks:
            new_insts = []
            for inst in blk.instructions:
                si = inst.sync_info
                lim = (1 if isinstance(inst, (mybir.InstDrain, mybir.InstNoOp,
                                              mybir.InstEventSemaphore))
                       else max_waits)
                if si is not None and si.on_wait and len(si.on_wait) > lim:
                    waits = list(si.on_wait)
                    extra, keep = waits[:-lim], waits[-lim:]
                    ci = 0
                    while extra:
                        chunk, extra = extra[:1], extra[1:]
                        new_insts.append(mybir.InstNoOp(
                            name=f"{inst.name}-waitsplit{ci}",
                            engine=inst.engine,
                            bass_nofuse=True,
                            sync_info=mybir.SyncInfo(on_wait=chunk, on_update=[]),
                        ))
                        ci += 1
                    si.on_wait = keep
                    n_split += 1
                new_insts.append(inst)
            blk.instructions = new_insts
    return n_split


# ----------------------------------------------------------------- the kernel
def build_nc(meta, enable_asserts=False, split_waits=True):
    n_tiles = meta["n_tiles"]
    per_core = meta["per_core"]
    npad = meta["npad"]
    nc = bass.Bass("TRN2", target_bir_lowering=False, debug=False,
                   enable_asserts=enable_asserts, num_swdge_queues=1)

    dt_ = nc.dram_tensor
    t_tab = dt_("table", [npad, 256], F16, kind="ExternalInput").ap()
    idxs_d = dt_("idxs", [128, n_tiles * 128], I16, kind="ExternalInput").ap()
    featsT = dt_("featsT", [C, per_core], F32, kind="ExternalInput").ap()
    actrE = dt_("actrE", [3, per_core * K], F16, kind="ExternalInput").ap()
    wk_d = dt_("wk", [C, C], F16, kind="ExternalInput").ap()
    wv_d = dt_("wv", [C, C], F16, kind="ExternalInput").ap()
    wqn_d = dt_("wqn", [C, C], F16, kind="ExternalInput").ap()
    wp2_d = dt_("wp2", [3, C], F16, kind="ExternalInput").ap()
    ww1s_d = dt_("ww1s", [C, CS], F16, kind="ExternalInput").ap()
    ww2r_d = dt_("ww2r", [CS, C], F16, kind="ExternalInput").ap()
    s_w_d = dt_("s_w", [C, 1], F32, kind="ExternalInput").ap()
    b_w_d = dt_("b_w", [C, 1], F32, kind="ExternalInput").ap()
    b1f_d = dt_("b1f", [CS, 1], F32, kind="ExternalInput").ap()
    be_d = dt_("be_bias", [C, 1], F32, kind="ExternalInput").ap()
    bvp_d = dt_("bvp", [C, 1], F32, kind="ExternalInput").ap()
    outT = dt_("outT", [C, per_core], F32, kind="ExternalOutput").ap()

    # gather base AP offset to row CO so signed indices reach the whole table
    t_base = t_tab[CO:npad, :]

    Relu = mybir.ActivationFunctionType.Relu
    Exp = mybir.ActivationFunctionType.Exp
    ADD = mybir.AluOpType.add
    MULT = mybir.AluOpType.mult
    SUB = mybir.AluOpType.subtract
    MAX = mybir.AluOpType.max

    nc.gpsimd.load_library(library_config.mlp)
    nidx_reg = nc.gpsimd.alloc_register("nidx")
    nc.gpsimd.reg_mov(nidx_reg, NPAIR)

    with tile.TileContext(nc) as tc:
        with (
            tc.tile_pool(name="const", bufs=1) as cpool,
            tc.tile_pool(name="gath", bufs=6) as gpool,
            tc.tile_pool(name="xs", bufs=2) as xpool,
            tc.tile_pool(name="front", bufs=2) as fpool,
            tc.tile_pool(name="rr", bufs=4) as rpool,
            tc.tile_pool(name="hh", bufs=4) as hpool,
            tc.tile_pool(name="slab", bufs=2) as epool,
            tc.tile_pool(name="tail", bufs=3) as tpool,
            tc.tile_pool(name="psWL", bufs=2, space="PSUM") as psW,
            tc.tile_pool(name="psV", bufs=3, space="PSUM") as psV,
            tc.tile_pool(name="psL", bufs=1, space="PSUM") as psL,
        ):
            # ---- constants into SBUF once
            def cload(ap_dram, shape, dtype, tag):
                t = cpool.tile(shape, dtype, tag=tag)
                nc.sync.dma_start(t[:], ap_dram)
                return t

            wk = cload(wk_d, [C, C], F16, "wk")
            wv = cload(wv_d, [C, C], F16, "wv")
            wqn = cload(wqn_d, [C, C], F16, "wqn")
            wp2 = cload(wp2_d, [3, C], F16, "wp2")
            ww1s = cload(ww1s_d, [C, CS], F16, "ww1s")
            ww2r = cload(ww2r_d, [CS, C], F16, "ww2r")
            s_w = cload(s_w_d, [C, 1], F32, "s_w")
            b_w = cload(b_w_d, [C, 1], F32, "b_w")
            b1f = cload(b1f_d, [CS, 1], F32, "b1f")
            be_b = cload(be_d, [C, 1], F32, "be_b")
            bvp = cload(bvp_d, [C, 1], F32, "bvp")
            ixs = cload(idxs_d, [128, n_tiles * 128], I16, "ixs")

            # whole-core featsT resident in SBUF (one DMA, 20KB/partition)
            ftw = cpool.tile([C, per_core], F32, tag="ftw")
            nc.sync.dma_start(ftw[:], featsT)

            ACHUNK = 2  # tiles per actrE load
            state = {}

            def s0_gather(t):
                cols = bass.ts(t, 128)
                g = gpool.tile([128, 2, NPAIR], F16, tag="g")
                nc.gpsimd.dma_gather(g[:], t_base, ixs[:, cols], NPAIR, nidx_reg,
                                     256, transpose=True, queue_num=0,
                                     single_packet=False)
                if t % ACHUNK == 0:
                    nch = min(ACHUNK, n_tiles - t)
                    act_ch = xpool.tile([3, ACHUNK * NPAIR], F16, tag="act")
                    nc.sync.dma_start(act_ch[:, :nch * NPAIR],
                                      actrE[:, t * NPAIR:(t + nch) * NPAIR])
                    state["act_ch"] = act_ch
                state[("g", t)] = (g, state["act_ch"])

            def s1_front(t):
                g, act_ch = state[("g", t)]
                act = act_ch[:, (t % ACHUNK) * NPAIR:(t % ACHUNK + 1) * NPAIR]
                xT16 = xpool.tile([C, 128], F16, tag="xT16")
                nc.scalar.copy(xT16[:], ftw[:, bass.ts(t, 128)])
                u = fpool.tile([3, NPAIR], F16, tag="u")
                nc.vector.tensor_tensor(u[:], g[0:3, 1, :], act, SUB)
                ru = fpool.tile([3, NPAIR], F16, tag="ru")
                nc.vector.tensor_scalar(ru[:], u[:], 0.0, None, MAX)
                state[("f", t)] = (xT16, ru)

            def s2_chunks(t):
                g, _ = state.pop(("g", t))
                xT16, ru = state.pop(("f", t))
                gf = g[:, 0, :]
                et = epool.tile([C, 2, NPAIR], F16, tag="et")
                e16 = et[:, 0, :]
                t2 = et[:, 1, :]
                filler = state.pop("tree_ops", [])

                def q_bcast(c):
                    p0 = c * (CHUNK // K)
                    return (xT16[:, p0:p0 + CHUNK // K]
                            .unsqueeze(2).broadcast_to([C, CHUNK // K, K]))

                # pair-granular pipeline: chunks (2p, 2p+1) share a
                # 2-bank w-psum and a 2-bank l-psum so the r / h2 / exp
                # activations each run once per pair at [.,1024] (halves
                # the ACT per-op init overhead). v stays per-chunk.
                def front2(p):
                    wpair = psW.tile([C, 2 * CHUNK], F32, tag="wl")
                    vv = []
                    for cc in range(2):
                        c = 2 * p + cc
                        csl = bass.ts(c, CHUNK)
                        wps = wpair[:, cc * CHUNK:(cc + 1) * CHUNK]
                        nc.tensor.matmul(wps, wk[:], gf[:, csl],
                                         start=True, stop=False)
                        nc.tensor.matmul(wps, wqn[:], q_bcast(c),
                                         start=False, stop=False)
                        nc.tensor.matmul(wps, wp2[:], ru[:, csl],
                                         start=False, stop=True)
                        vps = psV.tile([C, CHUNK], F32, tag="v")
                        nc.tensor.matmul(vps[:], wv[:], gf[:, csl],
                                         start=True, stop=False)
                        nc.tensor.matmul(vps[:], wp2[:], ru[:, csl],
                                         start=False, stop=True)
                        vv.append(vps)
                    r16 = rpool.tile([C, 2 * CHUNK], F16, tag="r")
                    nc.scalar.activation(r16[:], wpair[:], Relu,
                                         bias=b_w[:], scale=s_w[:])
                    return (wpair, vv, r16)

                def back2(p, wpair, vv, r16):
                    psl = bass.ts(p, 2 * CHUNK)
                    for cc in range(2):
                        nc.tensor.matmul(
                            wpair[0:CS, cc * CHUNK:(cc + 1) * CHUNK],
                            ww1s[:], r16[:, cc * CHUNK:(cc + 1) * CHUNK],
                            start=True, stop=True, skip_group_check=True)
                    h2 = hpool.tile([CS, 2 * CHUNK], F16, tag="h2")
                    nc.scalar.activation(h2[:], wpair[0:CS, :], Relu,
                                         bias=b1f[:])
                    lpair = psL.tile([C, 2 * CHUNK], F32, tag="l")
                    for cc in range(2):
                        nc.tensor.matmul(
                            lpair[:, cc * CHUNK:(cc + 1) * CHUNK], ww2r[:],
                            h2[:, cc * CHUNK:(cc + 1) * CHUNK],
                            start=True, stop=True)
                    nc.scalar.activation(e16[:, psl], lpair[:], Exp,
                                         bias=be_b[:])
                    for cc in range(2):
                        c = 2 * p + cc
                        csl = bass.ts(c, CHUNK)
                        nc.vector.tensor_tensor(t2[:, csl], e16[:, csl],
                                                vv[cc][:], MULT)
                    if filler:
                        filler.pop(0)()

                for p in range(2):
                    fr = front2(p)
                    back2(p, *fr)
                for op in filler:
                    op()
                state[("c", t)] = et

            def s3_trees(t):
                et = state.pop(("c", t))

                # one K-reduction tree over both planes (e sums -> S,
                # t2 sums -> aggU): 16->8->4->2->1, emitted as two closures
                # so s2 can interleave them into DVE bubbles
                st = {}

                def lv_a():
                    cur = et.rearrange("p q (a b) -> p q a b", b=K)
                    nx = tpool.tile([C, 2, 1024], F16, tag="tr16")
                    nxv = nx[:].rearrange("p q (a b) -> p q a b", b=8)
                    nc.vector.tensor_tensor(nxv, cur[:, :, :, 0:8],
                                            cur[:, :, :, 8:16], ADD)
                    st["c"] = nxv

                def lv_b():
                    cur = st["c"]
                    nx = tpool.tile([C, 2, 512], F16, tag="tr8")
                    nxv = nx[:].rearrange("p q (a b) -> p q a b", b=4)
                    nc.vector.tensor_tensor(nxv, cur[:, :, :, 0:4],
                                            cur[:, :, :, 4:8], ADD)
                    nx2 = tpool.tile([C, 2, 256], F16, tag="tr4")
                    nxv2 = nx2[:].rearrange("p q (a b) -> p q a b", b=2)
                    nc.vector.tensor_tensor(nxv2, nxv[:, :, :, 0:2],
                                            nxv[:, :, :, 2:4], ADD)
                    out32 = tpool.tile([C, 2, 128], F32, tag="trout")
                    nc.vector.tensor_tensor(out32[:], nxv2[:, :, :, 0],
                                            nxv2[:, :, :, 1], ADD)
                    state[("sa", t)] = out32

                state["tree_ops"] = [lv_a, lv_b]

            def s3_norm(t):
                for op in state.pop("tree_ops", []):
                    op()   # leftover tree work if s2 didn't run this iter
                sa = state.pop(("sa", t))
                S_t, aggU = sa[:, 0, :], sa[:, 1, :]
                rS = tpool.tile([C, 128], F32, tag="rS")
                nc.vector.reciprocal(rS[:], S_t)
                aggN = tpool.tile([C, 128], F32, tag="aggN")
                nc.gpsimd.tensor_tensor(aggN[:], aggU[:], rS[:], MULT)
                l1 = tpool.tile([C, 128], F32, tag="l1")
                nc.gpsimd.scalar_tensor_tensor(l1[:], aggN[:], bvp[:],
                                               ftw[:, bass.ts(t, 128)], ADD, ADD)
                outc = tpool.tile([C, 128], F32, tag="outc")
                nc.vector.scalar_tensor_tensor(outc[:], l1[:], 0.1, l1[:],
                                               MULT, MAX)
                nc.sync.dma_start(outT[:, bass.ts(t, 128)], outc[:])

            for i in range(n_tiles + 4):
                if 3 <= i < n_tiles + 3:
                    s3_trees(i - 3)
                if i < n_tiles:
                    s0_gather(i)
                if 1 <= i < n_tiles + 1:
                    s1_front(i - 1)
                if 2 <= i < n_tiles + 2:
                    s2_chunks(i - 2)
                if 4 <= i:
                    s3_norm(i - 4)

    from concourse.library_overlay import lower_extended_insts
    lower_extended_insts(nc)
    if split_waits:
        split_excess_waits(nc)
    return nc


# ------------------------------------------------------------- entry point
N_CORES = 8

_CACHE = {}


def kernel(**inputs) -> np.ndarray:
    """Full-input entry: shards points across 8 NeuronCores, runs the Bass
    kernel via run_bass_kernel_spmd, reassembles the full (1, N, C) output."""
    from concourse.bass_utils import run_bass_kernel_spmd

    xyz = np.asarray(inputs["xyz"], np.float32)
    feats = np.asarray(inputs["feats"], np.float32)
    nei = np.asarray(inputs["nei_ind"])
    params = {k: np.asarray(v, np.float32) for k, v in inputs.items()
              if k not in ("xyz", "feats", "nei_ind")}

    in_maps, meta = prep_inputs(xyz, feats, nei, params, N_CORES)

    key = (meta["n_tiles"], meta["per_core"], meta["npad"])
    if key not in _CACHE:
        _CACHE[key] = build_nc(meta)
    nc = _CACHE[key]

    res = run_bass_kernel_spmd(nc, in_maps, core_ids=list(range(N_CORES)))
    outs = [r["outT"] for r in res.results]          # each [C, per_core] f32
    full = np.concatenate(outs, axis=1).T             # [npad, C]
    return np.ascontiguousarray(full[None, :meta["n_real"]]).astype(np.float32)


# revision 59
# speedup vs baseline: 1.5186x; 1.0175x over previous
"""PointTransformerLayer Bass kernel for TRN2 (v4).

Design (per core, points sharded across 8 cores):
  - Packed DRAM table: one 512B row per point = [feats fp16 (128) | a fp16 (3)
    | zero pad], a = pos @ Afold (linear_p layer 1 + BN folded host-side).
  - SINGLE gather per tile: the gather's int16 indices are signed; the table
    base AP is offset to row CO=16384 so idx = j - CO spans [-16384, 24575],
    covering all 40960 rows in one gather (the gpsimd ucode sign-extends
    indices and the 32-bit address mul wraps correctly because the AP offset
    guarantees no underflow below the tensor base). The ucode trims trailing
    NEGATIVE indices, so the host guarantees each tile's last pair has
    idx >= 0 via a slot swap (or, in the astronomically rare fallback, by
    duplicating the needed row into a padding row >= CO).
  - Transpose-mode gather lands channels on partitions: G[c, (pt,k)].
  - All BN folded into weights / per-channel ACT scale+bias.
  - h-path (C -> C/S): single [16,CHUNK] matmul into the dead w-psum bank,
    relu+bias on ACT; single l-matmul per chunk (matmul inputs must sit at
    partition base 0 on this HW).
  - CHUNK=512, psum regions w(3)/v(2)/l(2) bufs, chunks software-pipelined;
    exp output IS the broadcast weight tile; t2 = e * vpsum in one DVE op;
    e16/t2 share one [C,2,NPAIR] slab so both K-reduction trees run as
    single DVE ops per level (16->8->4->2->1, f16 until the last level).
"""

import sys

sys.path.insert(0, "/opt/trn_rl_repo")
sys.path.insert(0, "/root/.axon_site/_ro/trn_rl_repo")

import numpy as np

import concourse.bass as bass
import concourse.tile as tile
from concourse import library_config, mybir

F16 = mybir.dt.float16
F32 = mybir.dt.float32
I16 = mybir.dt.int16

K = 16
C = 128
S = 8
CS = C // S  # 16
EPS = 1e-5
EXP_SHIFT = float(np.log(256.0))
PT_TILE = 128          # points per tile
NPAIR = PT_TILE * K    # 2048 gather columns per tile
CHUNK = 512            # psum column chunk (1 bank)
NCH = NPAIR // CHUNK   # 4 chunks per tile
HB = 4                 # h-blocks per chunk, at PE quadrant bases 0/32/64/96
HCOL = CHUNK // HB     # 128 pair-columns per h-block
CO = 16384             # table base-row offset: idx = j - CO (signed int16)


# ----------------------------------------------------------------- host math
def fold_params(p):
    """Fold BN params / biases. Tiny O(C^2) parameter-only preprocessing."""
    f32 = np.float32
    s_p = (p["p_gamma"] / np.sqrt(p["p_var"] + EPS)).astype(f32)
    Afold = (p["Wp1"] * s_p[None, :]).astype(f32)
    cfold = ((p["bp1"] - p["p_mean"]) * s_p + p["p_beta"]).astype(f32)

    s_w = (p["w_gamma"] / np.sqrt(p["w_var"] + EPS)).astype(f32)
    ball = (p["bk"] - p["bq"] + p["bp2"]).astype(f32)
    b_w = ((ball - p["w_mean"]) * s_w + p["w_beta"]).astype(f32)

    s1 = (p["w1_gamma"] / np.sqrt(p["w1_var"] + EPS)).astype(f32)
    ww1s = (p["Ww1"] * s1[None, :]).astype(np.float16)
    b1f = ((p["bw1"] - p["w1_mean"]) * s1 + p["w1_beta"]).astype(f32)

    ww2r = np.tile(p["Ww2"], (1, S)).astype(np.float16)          # [16, 128]
    be_bias = (np.tile(p["bw2"], S) - EXP_SHIFT).astype(f32)      # [128]
    bvp = (p["bv"] + p["bp2"]).astype(f32)                        # [128]

    return dict(
        wk=p["Wk"].astype(np.float16),
        wv=p["Wv"].astype(np.float16),
        wqn=(-p["Wq"]).astype(np.float16),
        wp2=p["Wp2"].astype(np.float16),  # [3, 128]
        ww1s=ww1s, ww2r=ww2r,
        s_w=s_w, b_w=b_w, b1f=b1f, be_bias=be_bias, bvp=bvp,
        Afold=Afold, cfold=cfold,
    )


def prep_inputs(xyz, feats, nei_ind, params, n_cores):
    """Build per-core in_maps. Host work is slicing / transposes / dtype
    conversion plus the tiny parameter folds above."""
    f = fold_params(params)
    n_real = feats.shape[1]
    per_core_raw = -(-n_real // n_cores)
    per_core = -(-per_core_raw // PT_TILE) * PT_TILE
    npad = per_core * n_cores
    n_tiles = per_core // PT_TILE

    feats0 = np.zeros((npad, C), np.float32)
    feats0[:n_real] = feats[0]
    pos0 = np.zeros((npad, 3), np.float32)
    pos0[:n_real] = xyz[0]
    ni = np.full((npad, K), CO, np.int64)   # padding points gather row CO
    ni[:n_real] = nei_ind[0]

    a = (pos0 @ f["Afold"]).astype(np.float32)            # [npad, 3]
    actrC = (a - f["cfold"][None, :]).astype(np.float32)  # center role

    # packed table rows: [feats f16 (128) | a f16 (3) | pad] = 256 f16 = 512B
    ent = np.zeros((npad, 256), np.float16)
    ent[:, :C] = feats0.astype(np.float16)
    ent[:, C:C + 3] = a.astype(np.float16)

    featsT = np.ascontiguousarray(feats0.T)               # [C, npad] f32
    actrT = np.ascontiguousarray(actrC.T)                 # [3, npad] f32

    def wrap_idx(arr_core):
        # arr_core: [per_core, K] int16 -> [128, n_tiles*128] in the
        # (s p)-wrapped layout dma_gather expects, replicated to 8 groups.
        out = np.zeros((128, n_tiles * 128), np.int16)
        for t in range(n_tiles):
            flat = arr_core[t * PT_TILE:(t + 1) * PT_TILE].reshape(-1)  # 2048
            w16 = flat.reshape(128, 16).T                                # [16,128]
            out[:, t * 128:(t + 1) * 128] = np.tile(w16, (8, 1))
        return out

    in_maps = []
    n_pad_rows = npad - n_real
    for c in range(n_cores):
        sl = slice(c * per_core, (c + 1) * per_core)
        ni_c = ni[sl].copy()                              # [per_core, K]
        table_c = ent                                     # shared unless patched
        # the gather ucode trims trailing negative indices: guarantee the
        # last pair of every tile has j >= CO (idx >= 0)
        for t in range(n_tiles):
            row = ni_c[t * PT_TILE + PT_TILE - 1]
            if row[K - 1] >= CO:
                continue
            hi = np.nonzero(row >= CO)[0]
            if len(hi):
                row[K - 1], row[hi[0]] = row[hi[0]], row[K - 1]
            else:
                # astronomically rare: duplicate the needed row into a
                # padding slot >= CO and retarget the index
                pr = n_real + (c * n_tiles + t) % n_pad_rows
                if table_c is ent:
                    table_c = ent.copy()
                table_c[pr] = ent[row[K - 1]]
                row[K - 1] = pr
        idxs = (ni_c - CO).astype(np.int16)
        actrE = np.repeat(actrT[:, sl].astype(np.float16), K, axis=1)  # [3, per_core*K]
        in_maps.append({
            "table": table_c,
            "idxs": wrap_idx(idxs),
            "featsT": np.ascontiguousarray(featsT[:, sl]),
            "actrE": np.ascontiguousarray(actrE),
            "wk": f["wk"], "wv": f["wv"], "wqn": f["wqn"], "wp2": f["wp2"],
            "ww1s": f["ww1s"], "ww2r": f["ww2r"],
            "s_w": f["s_w"].reshape(C, 1), "b_w": f["b_w"].reshape(C, 1),
            "b1f": f["b1f"].reshape(CS, 1),
            "be_bias": f["be_bias"].reshape(C, 1),
            "bvp": f["bvp"].reshape(C, 1),
        })
    meta = dict(n_tiles=n_tiles, per_core=per_core, npad=npad, n_real=n_real)
    return in_maps, meta


# ------------------------------------------------------------- walrus compat
def split_excess_waits(nc, max_waits=1):
    """This walrus build allows only 1 sync wait on CTRL instructions
    (Drain/NoOp) and a few on compute instructions. Move excess waits onto
    preceding single-wait NoOps."""
    n_split = 0
    for fn in nc.m.functions:
        for blk in fn.blocks:
            new_insts = []
            for inst in blk.instructions:
                si = inst.sync_info
                lim = (1 if isinstance(inst, (mybir.InstDrain, mybir.InstNoOp,
                                              mybir.InstEventSemaphore))
                       else max_waits)
                if si is not None and si.on_wait and len(si.on_wait) > lim:
                    waits = list(si.on_wait)
                    extra, keep = waits[:-lim], waits[-lim:]
                    ci = 0
                    while extra:
                        chunk, extra = extra[:1], extra[1:]
                        new_insts.append(mybir.InstNoOp(
                            name=f"{inst.name}-waitsplit{ci}",
                            engine=inst.engine,
                            bass_nofuse=True,
                            sync_info=mybir.SyncInfo(on_wait=chunk, on_update=[]),
                        ))
                        ci += 1
                    si.on_wait = keep
                    n_split += 1
                new_insts.append(inst)
            blk.instructions = new_insts
    return n_split


# ----------------------------------------------------------------- the kernel
def build_nc(meta, enable_asserts=False, split_waits=True):
    n_tiles = meta["n_tiles"]
    per_core = meta["per_core"]
    npad = meta["npad"]
    nc = bass.Bass("TRN2", target_bir_lowering=False, debug=False,
                   enable_asserts=enable_asserts, num_swdge_queues=1)

    dt_ = nc.dram_tensor
    t_tab = dt_("table", [npad, 256], F16, kind="ExternalInput").ap()
    idxs_d = dt_("idxs", [128, n_tiles * 128], I16, kind="ExternalInput").ap()
    featsT = dt_("featsT", [C, per_core], F32, kind="ExternalInput").ap()
    actrE = dt_("actrE", [3, per_core * K], F16, kind="ExternalInput").ap()
    wk_d = dt_("wk", [C, C], F16, kind="ExternalInput").ap()
    wv_d = dt_("wv", [C, C], F16, kind="ExternalInput").ap()
    wqn_d = dt_("wqn", [C, C], F16, kind="ExternalInput").ap()
    wp2_d = dt_("wp2", [3, C], F16, kind="ExternalInput").ap()
    ww1s_d = dt_("ww1s", [C, CS], F16, kind="ExternalInput").ap()
    ww2r_d = dt_("ww2r", [CS, C], F16, kind="ExternalInput").ap()
    s_w_d = dt_("s_w", [C, 1], F32, kind="ExternalInput").ap()
    b_w_d = dt_("b_w", [C, 1], F32, kind="ExternalInput").ap()
    b1f_d = dt_("b1f", [CS, 1], F32, kind="ExternalInput").ap()
    be_d = dt_("be_bias", [C, 1], F32, kind="ExternalInput").ap()
    bvp_d = dt_("bvp", [C, 1], F32, kind="ExternalInput").ap()
    outT = dt_("outT", [C, per_core], F32, kind="ExternalOutput").ap()

    # gather base AP offset to row CO so signed indices reach the whole table
    t_base = t_tab[CO:npad, :]

    Relu = mybir.ActivationFunctionType.Relu
    Exp = mybir.ActivationFunctionType.Exp
    ADD = mybir.AluOpType.add
    MULT = mybir.AluOpType.mult
    SUB = mybir.AluOpType.subtract
    MAX = mybir.AluOpType.max

    nc.gpsimd.load_library(library_config.mlp)
    nidx_reg = nc.gpsimd.alloc_register("nidx")
    nc.gpsimd.reg_mov(nidx_reg, NPAIR)

    with tile.TileContext(nc) as tc:
        with (
            tc.tile_pool(name="const", bufs=1) as cpool,
            tc.tile_pool(name="gath", bufs=6) as gpool,
            tc.tile_pool(name="xs", bufs=2) as xpool,
            tc.tile_pool(name="front", bufs=2) as fpool,
            tc.tile_pool(name="rr", bufs=4) as rpool,
            tc.tile_pool(name="hh", bufs=4) as hpool,
            tc.tile_pool(name="slab", bufs=2) as epool,
            tc.tile_pool(name="tail", bufs=3) as tpool,
            tc.tile_pool(name="psWL", bufs=2, space="PSUM") as psW,
            tc.tile_pool(name="psV", bufs=3, space="PSUM") as psV,
            tc.tile_pool(name="psL", bufs=1, space="PSUM") as psL,
        ):
            # ---- constants into SBUF once
            def cload(ap_dram, shape, dtype, tag):
                t = cpool.tile(shape, dtype, tag=tag)
                nc.sync.dma_start(t[:], ap_dram)
                return t

            wk = cload(wk_d, [C, C], F16, "wk")
            wv = cload(wv_d, [C, C], F16, "wv")
            wqn = cload(wqn_d, [C, C], F16, "wqn")
            wp2 = cload(wp2_d, [3, C], F16, "wp2")
            ww1s = cload(ww1s_d, [C, CS], F16, "ww1s")
            ww2r = cload(ww2r_d, [CS, C], F16, "ww2r")
            s_w = cload(s_w_d, [C, 1], F32, "s_w")
            b_w = cload(b_w_d, [C, 1], F32, "b_w")
            b1f = cload(b1f_d, [CS, 1], F32, "b1f")
            be_b = cload(be_d, [C, 1], F32, "be_b")
            bvp = cload(bvp_d, [C, 1], F32, "bvp")
            ixs = cload(idxs_d, [128, n_tiles * 128], I16, "ixs")

            # whole-core featsT resident in SBUF (one DMA, 20KB/partition)
            ftw = cpool.tile([C, per_core], F32, tag="ftw")
            nc.sync.dma_start(ftw[:], featsT)

            ACHUNK = 2  # tiles per actrE load
            state = {}

            def s0_gather(t):
                cols = bass.ts(t, 128)
                g = gpool.tile([128, 2, NPAIR], F16, tag="g")
                nc.gpsimd.dma_gather(g[:], t_base, ixs[:, cols], NPAIR, nidx_reg,
                                     256, transpose=True, queue_num=0,
                                     single_packet=False)
                if t % ACHUNK == 0:
                    nch = min(ACHUNK, n_tiles - t)
                    act_ch = xpool.tile([3, ACHUNK * NPAIR], F16, tag="act")
                    nc.sync.dma_start(act_ch[:, :nch * NPAIR],
                                      actrE[:, t * NPAIR:(t + nch) * NPAIR])
                    state["act_ch"] = act_ch
                state[("g", t)] = (g, state["act_ch"])

            def s1_front(t):
                g, act_ch = state[("g", t)]
                act = act_ch[:, (t % ACHUNK) * NPAIR:(t % ACHUNK + 1) * NPAIR]
                xT16 = xpool.tile([C, 128], F16, tag="xT16")
                nc.scalar.copy(xT16[:], ftw[:, bass.ts(t, 128)])
                u = fpool.tile([3, NPAIR], F16, tag="u")
                nc.vector.tensor_tensor(u[:], g[0:3, 1, :], act, SUB)
                ru = fpool.tile([3, NPAIR], F16, tag="ru")
                nc.vector.tensor_scalar(ru[:], u[:], 0.0, None, MAX)
                state[("f", t)] = (xT16, ru)

            def s2_chunks(t):
                g, _ = state.pop(("g", t))
                xT16, ru = state.pop(("f", t))
                gf = g[:, 0, :]
                et = epool.tile([C, 2, NPAIR], F16, tag="et")
                e16 = et[:, 0, :]
                t2 = et[:, 1, :]
                filler = state.pop("tree_ops", [])

                def q_bcast(c):
                    p0 = c * (CHUNK // K)
                    return (xT16[:, p0:p0 + CHUNK // K]
                            .unsqueeze(2).broadcast_to([C, CHUNK // K, K]))

                # pair-granular pipeline: chunks (2p, 2p+1) share a
                # 2-bank w-psum and a 2-bank l-psum so the r / h2 / exp
                # activations each run once per pair at [.,1024] (halves
                # the ACT per-op init overhead). v stays per-chunk.
                def front2(p):
                    wpair = psW.tile([C, 2 * CHUNK], F32, tag="wl")
                    vpair = psV.tile([C, 2 * CHUNK], F32, tag="v")
                    for cc in range(2):
                        c = 2 * p + cc
                        csl = bass.ts(c, CHUNK)
                        wps = wpair[:, cc * CHUNK:(cc + 1) * CHUNK]
                        nc.tensor.matmul(wps, wk[:], gf[:, csl],
                                         start=True, stop=False)
                        nc.tensor.matmul(wps, wqn[:], q_bcast(c),
                                         start=False, stop=False)
                        nc.tensor.matmul(wps, wp2[:], ru[:, csl],
                                         start=False, stop=True)
                        vps = vpair[:, cc * CHUNK:(cc + 1) * CHUNK]
                        nc.tensor.matmul(vps, wv[:], gf[:, csl],
                                         start=True, stop=False)
                        nc.tensor.matmul(vps, wp2[:], ru[:, csl],
                                         start=False, stop=True)
                    r16 = rpool.tile([C, 2 * CHUNK], F16, tag="r")
                    nc.scalar.activation(r16[:], wpair[:], Relu,
                                         bias=b_w[:], scale=s_w[:])
                    return (wpair, vpair, r16)

                def back2(p, wpair, vpair, r16):
                    psl = bass.ts(p, 2 * CHUNK)
                    for cc in range(2):
                        nc.tensor.matmul(
                            wpair[0:CS, cc * CHUNK:(cc + 1) * CHUNK],
                            ww1s[:], r16[:, cc * CHUNK:(cc + 1) * CHUNK],
                            start=True, stop=True, skip_group_check=True)
                    h2 = hpool.tile([CS, 2 * CHUNK], F16, tag="h2")
                    nc.scalar.activation(h2[:], wpair[0:CS, :], Relu,
                                         bias=b1f[:])
                    lpair = psL.tile([C, 2 * CHUNK], F32, tag="l")
                    for cc in range(2):
                        nc.tensor.matmul(
                            lpair[:, cc * CHUNK:(cc + 1) * CHUNK], ww2r[:],
                            h2[:, cc * CHUNK:(cc + 1) * CHUNK],
                            start=True, stop=True)
                    nc.scalar.activation(e16[:, psl], lpair[:], Exp,
                                         bias=be_b[:])
                    nc.vector.tensor_tensor(t2[:, psl], e16[:, psl],
                                            vpair[:], MULT)
                    if filler:
                        filler.pop(0)()

                for p in range(2):
                    fr = front2(p)
                    back2(p, *fr)
                for op in filler:
                    op()
                state[("c", t)] = et

            def s3_trees(t):
                et = state.pop(("c", t))

                # one K-reduction tree over both planes (e sums -> S,
                # t2 sums -> aggU): 16->8->4->2->1, emitted as two closures
                # so s2 can interleave them into DVE bubbles
                st = {}

                def lv_a():
                    cur = et.rearrange("p q (a b) -> p q a b", b=K)
                    nx = tpool.tile([C, 2, 1024], F16, tag="tr16")
                    nxv = nx[:].rearrange("p q (a b) -> p q a b", b=8)
                    nc.vector.tensor_tensor(nxv, cur[:, :, :, 0:8],
                                            cur[:, :, :, 8:16], ADD)
                    st["c"] = nxv

                def lv_b():
                    cur = st["c"]
                    nx = tpool.tile([C, 2, 512], F16, tag="tr8")
                    nxv = nx[:].rearrange("p q (a b) -> p q a b", b=4)
                    nc.vector.tensor_tensor(nxv, cur[:, :, :, 0:4],
                                            cur[:, :, :, 4:8], ADD)
                    nx2 = tpool.tile([C, 2, 256], F16, tag="tr4")
                    nxv2 = nx2[:].rearrange("p q (a b) -> p q a b", b=2)
                    nc.vector.tensor_tensor(nxv2, nxv[:, :, :, 0:2],
                                            nxv[:, :, :, 2:4], ADD)
                    out32 = tpool.tile([C, 2, 128], F32, tag="trout")
                    nc.vector.tensor_tensor(out32[:], nxv2[:, :, :, 0],
                                            nxv2[:, :, :, 1], ADD)
                    state[("sa", t)] = out32

                state["tree_ops"] = [lv_a, lv_b]

            def s3_norm(t):
                for op in state.pop("tree_ops", []):
                    op()   # leftover tree work if s2 didn't run this iter
                sa = state.pop(("sa", t))
                S_t, aggU = sa[:, 0, :], sa[:, 1, :]
                rS = tpool.tile([C, 128], F32, tag="rS")
                nc.vector.reciprocal(rS[:], S_t)
                aggN = tpool.tile([C, 128], F32, tag="aggN")
                nc.gpsimd.tensor_tensor(aggN[:], aggU[:], rS[:], MULT)
                l1 = tpool.tile([C, 128], F32, tag="l1")
                nc.gpsimd.scalar_tensor_tensor(l1[:], aggN[:], bvp[:],
                                               ftw[:, bass.ts(t, 128)], ADD, ADD)
                outc = tpool.tile([C, 128], F32, tag="outc")
                nc.vector.scalar_tensor_tensor(outc[:], l1[:], 0.1, l1[:],
                                               MULT, MAX)
                nc.sync.dma_start(outT[:, bass.ts(t, 128)], outc[:])

            for i in range(n_tiles + 4):
                if 3 <= i < n_tiles + 3:
                    s3_trees(i - 3)
                if i < n_tiles:
                    s0_gather(i)
                if 1 <= i < n_tiles + 1:
                    s1_front(i - 1)
                if 2 <= i < n_tiles + 2:
                    s2_chunks(i - 2)
                if 4 <= i:
                    s3_norm(i - 4)

    from concourse.library_overlay import lower_extended_insts
    lower_extended_insts(nc)
    if split_waits:
        split_excess_waits(nc)
    return nc


# ------------------------------------------------------------- entry point
N_CORES = 8

_CACHE = {}


def kernel(**inputs) -> np.ndarray:
    """Full-input entry: shards points across 8 NeuronCores, runs the Bass
    kernel via run_bass_kernel_spmd, reassembles the full (1, N, C) output."""
    from concourse.bass_utils import run_bass_kernel_spmd

    xyz = np.asarray(inputs["xyz"], np.float32)
    feats = np.asarray(inputs["feats"], np.float32)
    nei = np.asarray(inputs["nei_ind"])
    params = {k: np.asarray(v, np.float32) for k, v in inputs.items()
              if k not in ("xyz", "feats", "nei_ind")}

    in_maps, meta = prep_inputs(xyz, feats, nei, params, N_CORES)

    key = (meta["n_tiles"], meta["per_core"], meta["npad"])
    if key not in _CACHE:
        _CACHE[key] = build_nc(meta)
    nc = _CACHE[key]

    res = run_bass_kernel_spmd(nc, in_maps, core_ids=list(range(N_CORES)))
    outs = [r["outT"] for r in res.results]          # each [C, per_core] f32
    full = np.concatenate(outs, axis=1).T             # [npad, C]
    return np.ascontiguousarray(full[None, :meta["n_real"]]).astype(np.float32)
